# revision 15
# baseline (speedup 1.0000x reference)
"""DWA-CNN (DTW-aligned CNN) Trainium2 kernel, v2.

Problem: x[32,2048,128], w[3,128,8], b[8] -> out[32,2046,8]
out[b,p,f] = relu(b[f] + sum of dots along the DTW-optimal path between
window x[b,p:p+3,:] and filter w[:,:,f]).

Strategy (8 cores, pure data parallel over batch, 4 batches/core):
- Host pre-transposes x to [C=128, 8192] per core, split into an exact
  bf16 hi/lo pair (xh + xl == x in fp32), and prepares -2w in a matching
  bf16 hi/lo pair plus fp32 row norms nsr and weight norms/bias biasc.
- x loaded as 16 [C,1024] pieces alternating between the sync (HWDGE)
  and gpsimd (SWDGE) DMA queues; per 512-pos block: 3 bf16 matmuls
  (wh*xh + wh*xl + wl*xh in fp32 PSUM == exact fp32 -2*dots).
- PSUM evacuated per block by ScalarE/VectorE (alternating) into a
  [24, 8192] staging tile; 48 small SBUF->SBUF DMAs (rotating over the
  3 DMA-capable engines) scatter rows into the [(q*8+f) partition,
  j*JW+pos] layout (partition-offset copies; no DRAM bounce).
- E = Gm + nS via one broadcast 3-window TT on VectorE; D = sqrt(E+nW)
  in-place on ScalarE.
- DTW DP exploits D>=0: at cell (2,2) the diagonal always wins (ties
  break diag-first), which deletes the KA/LA/X1 machinery of the naive
  backtrack; c13/c31 are only needed for the LB/LC masks.  The whole
  DP is ~16 fused two-window ops on VectorE.
- Final relu(-0.5*acc + b) on ScalarE; result [128,512] DMAed out; host
  re-assembles [32,2046,8].
"""
import numpy as np

B, T, C, K, F = 32, 2048, 128, 3, 8
P = T - K + 1            # 2046
NCORES = 8
NB = B // NCORES         # batches per core
TL = NB * T              # 8192 positions per core
FD = 512
NQ = TL // FD            # 16 q blocks
JW = FD + 4              # per-j stride in the concatenated arrays

_cache = {}


def _build_program():
    import concourse.tile as tile
    from concourse import bacc, mybir

    f32 = mybir.dt.float32
    bf16 = mybir.dt.bfloat16
    u32 = mybir.dt.uint32
    Alu = mybir.AluOpType
    Act = mybir.ActivationFunctionType

    nc = bacc.Bacc(
        "TRN2",
        target_bir_lowering=False,
        debug=False,
        enable_asserts=False,
        num_devices=NCORES,
    )

    xh = nc.dram_tensor("xh", [C, TL], bf16, kind="ExternalInput").ap()
    xl = nc.dram_tensor("xl", [C, TL], bf16, kind="ExternalInput").ap()
    wcat = nc.dram_tensor("wcat", [C, 48], bf16, kind="ExternalInput").ap()
    nsr = nc.dram_tensor("nsr", [C, JW], f32, kind="ExternalInput").ap()
    biasc = nc.dram_tensor("biasc", [C, 4], f32, kind="ExternalInput").ap()
    res = nc.dram_tensor("res", [C, FD], f32, kind="ExternalOutput").ap()

    from contextlib import ExitStack

    with tile.TileContext(nc) as tc, ExitStack() as ctx:
        const = ctx.enter_context(tc.tile_pool(name="const", bufs=1))
        xin = ctx.enter_context(tc.tile_pool(name="xin", bufs=1))
        psum = ctx.enter_context(tc.tile_pool(name="psum", bufs=4,
                                              space="PSUM"))
        stage = ctx.enter_context(tc.tile_pool(name="stage", bufs=1))
        arrs = ctx.enter_context(tc.tile_pool(name="arrs", bufs=1))
        work = ctx.enter_context(tc.tile_pool(name="work", bufs=1))

        HW2 = FD + 2

        wcat_sb = const.tile([C, 48], bf16, tag="wcat")
        bias_sb = const.tile([C, 4], f32, tag="bias")
        nsr_sb = const.tile([C, JW], f32, tag="nsr")
        scr = const.tile([C, 4], f32, tag="scr")
        nc.scalar.dma_start(wcat_sb[:], wcat)
        nc.scalar.dma_start(bias_sb[:], biasc)
        # dummy ACT ops to pull the activation table loads off the tail
        nc.scalar.activation(scr[:, 0:1], bias_sb[:, 3:4], Act.Sqrt)
        nc.scalar.activation(scr[:, 1:2], bias_sb[:, 3:4], Act.Relu)
        nc.scalar.copy(scr[:, 2:3], bias_sb[:, 3:4])

        # x pieces: [C, 2048] each, 4 per tensor; piece 0 lands as two
        # halves so the first matmuls start as early as possible
        NP = 4
        PW = TL // NP        # 2048
        xh_t = []
        xl_t = []
        for i in range(NP):
            th = xin.tile([C, PW], bf16, tag=f"xh{i}", name=f"xh{i}")
            tl_ = xin.tile([C, PW], bf16, tag=f"xl{i}", name=f"xl{i}")
            xh_t.append(th)
            xl_t.append(tl_)
        # keep per-queue issue order == global piece order so the PE streams
        H = PW // 2
        nc.sync.dma_start(xh_t[0][:, 0:H], xh[:, 0:H])
        nc.gpsimd.dma_start(xl_t[0][:, 0:H], xl[:, 0:H])
        nc.scalar.dma_start(xh_t[0][:, H:PW], xh[:, H:PW])
        nc.scalar.dma_start(xl_t[0][:, H:PW], xl[:, H:PW])
        nc.sync.dma_start(xh_t[1][:], xh[:, PW:2 * PW])
        nc.gpsimd.dma_start(xl_t[1][:], xl[:, PW:2 * PW])
        nc.sync.dma_start(xh_t[2][:], xh[:, 2 * PW:3 * PW])
        nc.gpsimd.dma_start(xl_t[2][:], xl[:, 2 * PW:3 * PW])
        nc.sync.dma_start(xh_t[3][:], xh[:, 3 * PW:4 * PW])
        nc.gpsimd.dma_start(xl_t[3][:], xl[:, 3 * PW:4 * PW])
        nc.gpsimd.dma_start(nsr_sb[:], nsr)

        # staging for all 16 q blocks (fp32 rows j*8+f)
        stg = stage.tile([24, TL + 4], f32, tag="stg")
        # zero the 2-col tail so q=15's halo lands as 0
        nc.vector.memset(stg[0:24, TL:TL + 2], 0.0)

        # scattered arrays [partition f*16+q, j*JW + pos]
        Gm_all = arrs.tile([C, K * JW], f32, tag="gm")
        Dj_all = arrs.tile([C, K * JW], f32, tag="dj")

        wh = wcat_sb[:, 0:24]
        wl = wcat_sb[:, 24:48]

        # per-unit (1024 cols = 2 blocks) matmul + evac; matmuls are 512 wide
        # (PSUM bank limit) but evac is per 1024-col unit
        NU = NQ // 2
        UW = 2 * FD
        for u in range(NU):
            pc = u // 2              # x piece index
            ps = psum.tile([24, UW], f32, tag="ps", name=f"ps{u}")
            for h in range(2):
                xs = slice((u % 2) * UW + h * FD, (u % 2) * UW + (h + 1) * FD)
                po = slice(h * FD, (h + 1) * FD)
                nc.tensor.matmul(ps[:, po], wh, xh_t[pc][:, xs],
                                 start=True, stop=False)
                nc.tensor.matmul(ps[:, po], wh, xl_t[pc][:, xs],
                                 start=False, stop=False)
                nc.tensor.matmul(ps[:, po], wl, xh_t[pc][:, xs],
                                 start=False, stop=True)
            dst = stg[:, u * UW:(u + 1) * UW]
            if u == NU - 1:
                # split the last evac across both engines to shorten the tail
                nc.scalar.copy(stg[:, u * UW:u * UW + FD], ps[:, 0:FD])
                nc.vector.tensor_scalar_add(
                    stg[:, u * UW + FD:(u + 1) * UW], ps[:, FD:UW], 0.0)
            elif u % 2 == 0:
                nc.scalar.copy(dst, ps[:])
            else:
                nc.vector.tensor_scalar_add(dst, ps[:], 0.0)

        # scatter: with p = f*16+q the (f, q) iteration order equals the
        # natural dst partition order, so one SBUF->SBUF DMA per (j, col
        # half) does the partition fold: stg[j*8+f, q*512+col] -> dst[p,
        # col].  Column halves interleave the 3 js across the queues so the
        # SDMA engines keep all 6 source SBUF ports busy.
        engs = [nc.sync, nc.scalar, nc.gpsimd]
        CH2 = HW2 // 2       # 257
        for h in range(2):
            for j in range(K):
                c0 = h * CH2
                c1 = HW2 if h == 1 else CH2
                s = stg[j * 8:(j + 1) * 8, c0:c1]
                v = s.unsqueeze(1)
                v.ap[1] = [FD, NQ]
                engs[(j + h) % 3].dma_start(
                    Gm_all[:, j * JW + c0:j * JW + c1], v)

        V = nc.vector
        TT = V.tensor_tensor
        CP = V.copy_predicated

        # E = Gm + nS (broadcast nsr over the 3 j-blocks), one fused op
        HW2 = FD + 2
        ev = Dj_all[:].rearrange("p (j e) -> p j e", j=K)[:, :, 0:HW2]
        gv = Gm_all[:].rearrange("p (j e) -> p j e", j=K)[:, :, 0:HW2]
        nv = nsr_sb[:, 0:HW2].unsqueeze(1)
        nv.ap[1] = [0, K]
        TT(ev, gv, nv, Alu.add)

        # D = sqrt(E + nW_j), in place
        for j in range(K):
            sl = slice(j * JW, j * JW + HW2)
            nc.scalar.activation(Dj_all[:, sl], Dj_all[:, sl], Act.Sqrt,
                                 bias=bias_sb[:, j:j + 1])

        # work tile slots
        (S_c12, S_c21, S_c13, S_c31, S_c22, S_mn2, S_mn3, S_c23, S_c32,
         S_mbcD, S_s10, S_s01, S_s20, S_s02, S_U, S_X2, S_XV1, S_XV2,
         S_ACCM, S_res) = range(20)
        NW = 20
        W = work.tile([C, NW * FD], f32, tag="W")
        M = work.tile([C, 6 * FD], u32, tag="M")
        M_KB, M_KC, M_LB, M_LC, M_KD, M_LD = range(6)

        def w1(s):
            return W[:, s * FD:(s + 1) * FD]

        def m1(s):
            return M[:, s * FD:(s + 1) * FD]

        def win2(ap2d, off_a, off_b, n=FD):
            v = ap2d[:, off_a:off_a + n].unsqueeze(1)
            v.ap[1] = [off_b - off_a, 2]
            return v

        def wwin(sa, sb):
            return win2(W[:], sa * FD, sb * FD)

        def mwin(sa, sb):
            return win2(M[:], sa * FD, sb * FD)

        def dwin(ia, ja, ib, jb):
            return win2(Dj_all[:], ja * JW + ia, jb * JW + ib)

        def gwin(ia, ja, ib, jb):
            return win2(Gm_all[:], ja * JW + ia, jb * JW + ib)

        def dd(i, j):
            return Dj_all[:, j * JW + i:j * JW + i + FD]

        def gg(i, j):
            return Gm_all[:, j * JW + i:j * JW + i + FD]

        # g-sums first (only need Gm; overlap ACT sqrt)
        # [s10|s01] = [g10|g01] + g00
        TT(wwin(S_s10, S_s01), gwin(1, 0, 0, 1), gwin(0, 0, 0, 0), Alu.add)
        # [s20|s02] = [g20|g02] + [s10|s01]
        TT(wwin(S_s20, S_s02), gwin(2, 0, 0, 2), wwin(S_s10, S_s01), Alu.add)
        # U = g11 + g00
        TT(w1(S_U), gg(1, 1), gg(0, 0), Alu.add)
        # X2/X3 defaults (ACT): X2 = U; X3 lives in the s02 slot in-place
        nc.scalar.copy(w1(S_X2), w1(S_U))

        # cost chain.  c11 = D00; diag always wins at (2,2) since D >= 0.
        # [c12|c21] = [D01|D10] + c11
        TT(wwin(S_c12, S_c21), dwin(0, 1, 1, 0), dwin(0, 0, 0, 0), Alu.add)
        # c22 = D11 + c11
        TT(w1(S_c22), dd(1, 1), dd(0, 0), Alu.add)
        # [c13|c31] = [D02|D20] + [c12|c21]   (only feeds LB/LC)
        TT(wwin(S_c13, S_c31), dwin(0, 2, 2, 0), wwin(S_c12, S_c21), Alu.add)
        # [mn2|mn3] = min([c12|c21], c22)   (c13>=c12, c31>=c21 drop out)
        TT(wwin(S_mn2, S_mn3), wwin(S_c12, S_c21), wwin(S_c22, S_c22), Alu.min)
        # [c23|c32] = [D12|D21] + [mn2|mn3]
        TT(wwin(S_c23, S_c32), dwin(1, 2, 2, 1), wwin(S_mn2, S_mn3), Alu.add)
        # [KB|KC] = [c12|c21] <= c22
        TT(mwin(M_KB, M_KC), wwin(S_c12, S_c21), wwin(S_c22, S_c22), Alu.is_le)
        # [LB|LC] = [c22|c31] <= [c13|c22]
        TT(mwin(M_LB, M_LC), wwin(S_c22, S_c31), wwin(S_c13, S_c22), Alu.is_le)
        # mbcD = min(c32, c23)
        TT(w1(S_mbcD), w1(S_c32), w1(S_c23), Alu.min)
        # [KD|LD] = [c22|c32] <= [mbcD|c23]
        TT(mwin(M_KD, M_LD), wwin(S_c22, S_c32), wwin(S_mbcD, S_c23),
           Alu.is_le)

        # X2 = KC ? s10 : (LC ? s20 : U);  X3 = KB ? s01 : (LB ? U : s02)
        CP(wwin(S_X2, S_s02), mwin(M_LC, M_LB), wwin(S_s20, S_U))
        CP(wwin(S_X2, S_s02), mwin(M_KC, M_KB), wwin(S_s10, S_s01))
        # [XV1|XV2] = [g21|g12] + [X2|X3]
        TT(wwin(S_XV1, S_XV2), gwin(2, 1, 1, 2), wwin(S_X2, S_s02), Alu.add)
        # X4 = KD ? U : (LD ? XV1 : XV2), built in place in XV2
        CP(w1(S_XV2), m1(M_LD), w1(S_XV1))
        CP(w1(S_XV2), m1(M_KD), w1(S_U))
        # ACCM = g22 + X4
        TT(w1(S_ACCM), gg(2, 2), w1(S_XV2), Alu.add)

        nc.scalar.activation(
            w1(S_res), w1(S_ACCM), Act.Relu, bias=bias_sb[:, 3:4], scale=-0.5)
        nc.scalar.dma_start(res, w1(S_res))

    nc.compile()
    return nc


def _host_prep(x, w, b):
    """Build per-core input maps."""
    import ml_dtypes

    x = np.ascontiguousarray(np.asarray(x, np.float32))
    w = np.asarray(w, np.float32)
    b = np.asarray(b, np.float32)

    w2m = np.zeros((C, 24), np.float32)
    for j in range(K):
        for f in range(F):
            w2m[:, j * 8 + f] = -2.0 * w[j, :, f]
    wh = w2m.astype(ml_dtypes.bfloat16)
    wlo = (w2m - wh.astype(np.float32)).astype(ml_dtypes.bfloat16)
    wcat = np.concatenate([wh, wlo], axis=1)              # [C, 48] bf16

    nW = (w ** 2).sum(1)                                  # [K, F]
    biasc = np.zeros((C, 4), np.float32)
    for q in range(NQ):
        for f in range(F):
            for j in range(K):
                biasc[f * NQ + q, j] = nW[j, f]
            biasc[f * NQ + q, 3] = b[f]

    in_maps = []
    for r in range(NCORES):
        x4 = x[r * NB:(r + 1) * NB]                       # [NB,T,C]
        flat = x4.reshape(TL, C)
        xT = np.ascontiguousarray(flat.T)                 # [C, TL] fp32
        xhh = xT.astype(ml_dtypes.bfloat16)
        xll = (xT - xhh.astype(np.float32)).astype(ml_dtypes.bfloat16)
        nS = np.einsum("tc,tc->t", flat, flat).astype(np.float32)
        nsr = np.ones((C, JW), np.float32)
        for q in range(NQ):
            lo = q * FD
            hi = min(TL, lo + FD + 2)
            nsr[q::NQ, 0:hi - lo] = nS[lo:hi][None, :]
        in_maps.append({
            "xh": xhh, "xl": xll, "wcat": wcat, "nsr": nsr, "biasc": biasc,
        })
    return in_maps


def _assemble(results):
    out = np.empty((B, P, F), np.float32)
    for r in range(NCORES):
        resr = results[r]["res"]                          # [128, 512]
        arr = resr.reshape(F, NQ, FD)                     # [f, q, p_lo]
        for f in range(F):
            series = arr[f].reshape(TL).reshape(NB, T)
            out[r * NB:(r + 1) * NB, :, f] = series[:, :P]
    return out


def kernel(x, w, b):
    from concourse.bass_utils import run_bass_kernel_spmd

    if "nc" not in _cache:
        _cache["nc"] = _build_program()
    nc = _cache["nc"]
    in_maps = _host_prep(x, w, b)
    out = run_bass_kernel_spmd(nc, in_maps, core_ids=list(range(NCORES)))
    return _assemble(out.results)


if __name__ == "__main__":
    rng = np.random.default_rng(0)
    x = rng.standard_normal((B, T, C), dtype=np.float32)
    w = (rng.standard_normal((K, C, F)) * 0.1).astype(np.float32)
    b = np.zeros((F,), np.float32)
    o = kernel(x, w, b)
    print("kernel ran, out shape", o.shape, float(np.abs(o).sum()))


# revision 16
# speedup vs baseline: 1.1441x; 1.1441x over previous
"""DWA-CNN (DTW-aligned CNN) Trainium2 kernel.

Problem: x[32,2048,128], w[3,128,8], b[8] -> out[32,2046,8]
out[b,p,f] = relu(b[f] + sum of dots along the DTW-optimal path between
window x[b,p:p+3,:] and filter w[:,:,f]).

Strategy (8 cores, pure data parallel over batch, 4 batches/core):
- Host pre-transposes x to [C=128, 8192] per core, split into an exact
  bf16 hi/lo pair (xh + xl == x in fp32), plus -2w as a bf16 hi/lo pair
  and fp32 row norms nsr / weight norms+bias biasc.
- x loaded as 16 [C,1024] pieces alternating sync/gpsimd DMA queues;
  per 512-pos block: 3 bf16 matmuls (wh*xh + wh*xl + wl*xh in fp32
  PSUM == exact fp32 -2*dots).
- PSUM evacuated per block by ScalarE/VectorE (alternating) into a
  [24, 8192] staging tile; 48 small SBUF->SBUF DMAs (rotating over the
  3 DMA-capable engines, issued incrementally as blocks finish)
  scatter rows into the [(q*8+f) partition, j*JW+pos] layout.
- E = Gm + nS via one broadcast 3-window TT on VectorE; D = sqrt(E+nW)
  in-place on ScalarE (activation tables preloaded early).
- DTW DP exploits D>=0: at cell (2,2) the diagonal always wins (ties
  break diag-first), deleting the KA/LA/X1 machinery; c13/c31 are only
  needed for the LB/LC masks.  ~16 fused two-window ops on VectorE.
- Final relu(-0.5*acc + b) on ScalarE; result [128,512] DMAed out; host
  re-assembles [32,2046,8].
"""
import numpy as np

B, T, C, K, F = 32, 2048, 128, 3, 8
P = T - K + 1            # 2046
NCORES = 8
NB = B // NCORES         # batches per core
TL = NB * T              # 8192 positions per core
FD = 512
NQ = TL // FD            # 16 q blocks
JW = FD + 4              # per-j stride in the concatenated arrays

_cache = {}


def _build_program():
    import concourse.tile as tile
    from concourse import bacc, mybir

    f32 = mybir.dt.float32
    bf16 = mybir.dt.bfloat16
    u32 = mybir.dt.uint32
    Alu = mybir.AluOpType
    Act = mybir.ActivationFunctionType

    nc = bacc.Bacc(
        "TRN2",
        target_bir_lowering=False,
        debug=False,
        enable_asserts=False,
        num_devices=NCORES,
    )

    xh = nc.dram_tensor("xh", [C, TL], bf16, kind="ExternalInput").ap()
    xl = nc.dram_tensor("xl", [C, TL], bf16, kind="ExternalInput").ap()
    wcat = nc.dram_tensor("wcat", [C, 48], bf16, kind="ExternalInput").ap()
    nsr = nc.dram_tensor("nsr", [C, JW], f32, kind="ExternalInput").ap()
    biasc = nc.dram_tensor("biasc", [C, 4], f32, kind="ExternalInput").ap()
    res = nc.dram_tensor("res", [C, FD], f32, kind="ExternalOutput").ap()

    from contextlib import ExitStack

    with tile.TileContext(nc) as tc, ExitStack() as ctx:
        const = ctx.enter_context(tc.tile_pool(name="const", bufs=1))
        xin = ctx.enter_context(tc.tile_pool(name="xin", bufs=1))
        psum = ctx.enter_context(tc.tile_pool(name="psum", bufs=6,
                                              space="PSUM"))
        stage = ctx.enter_context(tc.tile_pool(name="stage", bufs=1))
        arrs = ctx.enter_context(tc.tile_pool(name="arrs", bufs=1))
        work = ctx.enter_context(tc.tile_pool(name="work", bufs=1))

        dmaengs = [nc.sync, nc.scalar, nc.gpsimd]
        _dmac = [0]

        def dma3(dst, src):
            e = dmaengs[_dmac[0] % 3]
            _dmac[0] += 1
            e.dma_start(dst, src)

        wcat_sb = const.tile([C, 48], bf16, tag="wcat")
        bias_sb = const.tile([C, 4], f32, tag="bias")
        nsr_sb = const.tile([C, JW], f32, tag="nsr")
        scr = const.tile([C, 4], f32, tag="scr")
        nc.scalar.dma_start(wcat_sb[:], wcat)
        nc.scalar.dma_start(bias_sb[:], biasc)
        # dummy ACT ops to pull the activation table loads off the tail
        nc.scalar.activation(scr[:, 0:1], bias_sb[:, 3:4], Act.Sqrt)
        nc.scalar.activation(scr[:, 1:2], bias_sb[:, 3:4], Act.Relu)
        nc.scalar.copy(scr[:, 2:3], bias_sb[:, 3:4])

        # x pieces: [C, 1024] each, 8 per tensor, alternating sync/gpsimd
        NP = 8
        PW = TL // NP        # 1024
        xh_t = []
        xl_t = []
        for i in range(NP):
            th = xin.tile([C, PW], bf16, tag=f"xh{i}", name=f"xh{i}")
            tl_ = xin.tile([C, PW], bf16, tag=f"xl{i}", name=f"xl{i}")
            xh_t.append(th)
            xl_t.append(tl_)
        for i in range(NP):
            sl = slice(i * PW, (i + 1) * PW)
            (nc.sync if i % 2 == 0 else nc.gpsimd).dma_start(
                xh_t[i][:], xh[:, sl])
            (nc.gpsimd if i % 2 == 0 else nc.sync).dma_start(
                xl_t[i][:], xl[:, sl])
        nc.gpsimd.dma_start(nsr_sb[:], nsr)

        # staging for all 16 q blocks (fp32 rows j*8+f)
        stg = stage.tile([24, TL + 4], f32, tag="stg")

        # scattered arrays [partition q*8+f, j*JW + pos]
        Gm_all = arrs.tile([C, K * JW], f32, tag="gm")
        Dj_all = arrs.tile([C, K * JW], f32, tag="dj")

        # halo tails for q=15 (positions beyond TL) default 0; engine APs
        # need 32-aligned partition base, so cover q=12..15 (q<15 halos are
        # later overwritten by their scatter DMAs)
        for j in range(K):
            nc.vector.memset(Gm_all[96:128, j * JW + FD:j * JW + FD + 2], 0.0)

        wh = wcat_sb[:, 0:24]
        wl = wcat_sb[:, 24:48]

        # per-block matmul + evac + incremental scatter
        for b in range(NQ):
            pc = b // 2              # x piece index
            xs = slice((b % 2) * FD, (b % 2 + 1) * FD)
            ps = psum.tile([24, FD], f32, tag="ps", name=f"ps{b}")
            nc.tensor.matmul(ps[:], wh, xh_t[pc][:, xs], start=True, stop=False)
            nc.tensor.matmul(ps[:], wh, xl_t[pc][:, xs], start=False, stop=False)
            nc.tensor.matmul(ps[:], wl, xh_t[pc][:, xs], start=False, stop=True)
            dst = stg[:, b * FD:(b + 1) * FD]
            if b % 2 == 0:
                nc.scalar.copy(dst, ps[:])
            else:
                nc.vector.tensor_scalar_add(dst, ps[:], 0.0)
            # scatter for block b-1 (needs halo = first 2 cols of block b)
            if b > 0:
                q = b - 1
                for j in range(K):
                    dma3(Gm_all[q * 8:(q + 1) * 8, j * JW:j * JW + FD + 2],
                         stg[j * 8:(j + 1) * 8, q * FD:(q + 1) * FD + 2])
        q = NQ - 1
        for j in range(K):
            dma3(Gm_all[q * 8:(q + 1) * 8, j * JW:j * JW + FD],
                 stg[j * 8:(j + 1) * 8, q * FD:(q + 1) * FD])

        V = nc.vector
        TT = V.tensor_tensor
        CP = V.copy_predicated

        # E = Gm + nS (broadcast nsr over the 3 j-blocks), one fused op
        HW2 = FD + 2
        ev = Dj_all[:].rearrange("p (j e) -> p j e", j=K)[:, :, 0:HW2]
        gv = Gm_all[:].rearrange("p (j e) -> p j e", j=K)[:, :, 0:HW2]
        nv = nsr_sb[:, 0:HW2].unsqueeze(1)
        nv.ap[1] = [0, K]
        TT(ev, gv, nv, Alu.add)

        # D = sqrt(E + nW_j), in place
        for j in range(K):
            sl = slice(j * JW, j * JW + HW2)
            nc.scalar.activation(Dj_all[:, sl], Dj_all[:, sl], Act.Sqrt,
                                 bias=bias_sb[:, j:j + 1])

        # work tile slots
        (S_c12, S_c21, S_c13, S_c31, S_c22, S_mn2, S_mn3, S_c23, S_c32,
         S_mbcD, S_s10, S_s01, S_s20, S_s02, S_U, S_X2, S_XV1, S_XV2,
         S_ACCM, S_res) = range(20)
        NW = 20
        W = work.tile([C, NW * FD], f32, tag="W")
        M = work.tile([C, 6 * FD], u32, tag="M")
        M_KB, M_KC, M_LB, M_LC, M_KD, M_LD = range(6)

        def w1(s):
            return W[:, s * FD:(s + 1) * FD]

        def m1(s):
            return M[:, s * FD:(s + 1) * FD]

        def win2(ap2d, off_a, off_b, n=FD):
            v = ap2d[:, off_a:off_a + n].unsqueeze(1)
            v.ap[1] = [off_b - off_a, 2]
            return v

        def wwin(sa, sb):
            return win2(W[:], sa * FD, sb * FD)

        def mwin(sa, sb):
            return win2(M[:], sa * FD, sb * FD)

        def dwin(ia, ja, ib, jb):
            return win2(Dj_all[:], ja * JW + ia, jb * JW + ib)

        def gwin(ia, ja, ib, jb):
            return win2(Gm_all[:], ja * JW + ia, jb * JW + ib)

        def dd(i, j):
            return Dj_all[:, j * JW + i:j * JW + i + FD]

        def gg(i, j):
            return Gm_all[:, j * JW + i:j * JW + i + FD]

        # g-sums first (only need Gm; overlap ACT sqrt)
        TT(wwin(S_s10, S_s01), gwin(1, 0, 0, 1), gwin(0, 0, 0, 0), Alu.add)
        TT(wwin(S_s20, S_s02), gwin(2, 0, 0, 2), wwin(S_s10, S_s01), Alu.add)
        TT(w1(S_U), gg(1, 1), gg(0, 0), Alu.add)
        # X2 default (ACT); X3 lives in the s02 slot in-place
        nc.scalar.copy(w1(S_X2), w1(S_U))

        # cost chain.  c11 = D00; diag always wins at (2,2) since D >= 0.
        TT(wwin(S_c12, S_c21), dwin(0, 1, 1, 0), dwin(0, 0, 0, 0), Alu.add)
        TT(w1(S_c22), dd(1, 1), dd(0, 0), Alu.add)
        TT(wwin(S_c13, S_c31), dwin(0, 2, 2, 0), wwin(S_c12, S_c21), Alu.add)
        TT(wwin(S_mn2, S_mn3), wwin(S_c12, S_c21), wwin(S_c22, S_c22), Alu.min)
        TT(wwin(S_c23, S_c32), dwin(1, 2, 2, 1), wwin(S_mn2, S_mn3), Alu.add)
        TT(mwin(M_KB, M_KC), wwin(S_c12, S_c21), wwin(S_c22, S_c22), Alu.is_le)
        TT(mwin(M_LB, M_LC), wwin(S_c22, S_c31), wwin(S_c13, S_c22), Alu.is_le)
        TT(w1(S_mbcD), w1(S_c32), w1(S_c23), Alu.min)
        TT(mwin(M_KD, M_LD), wwin(S_c22, S_c32), wwin(S_mbcD, S_c23),
           Alu.is_le)

        # X2 = KC ? s10 : (LC ? s20 : U);  X3 = KB ? s01 : (LB ? U : s02)
        CP(wwin(S_X2, S_s02), mwin(M_LC, M_LB), wwin(S_s20, S_U))
        CP(wwin(S_X2, S_s02), mwin(M_KC, M_KB), wwin(S_s10, S_s01))
        TT(wwin(S_XV1, S_XV2), gwin(2, 1, 1, 2), wwin(S_X2, S_s02), Alu.add)
        # X4 = KD ? U : (LD ? XV1 : XV2), built in place in XV2
        CP(w1(S_XV2), m1(M_LD), w1(S_XV1))
        CP(w1(S_XV2), m1(M_KD), w1(S_U))
        TT(w1(S_ACCM), gg(2, 2), w1(S_XV2), Alu.add)

        nc.scalar.activation(
            w1(S_res), w1(S_ACCM), Act.Relu, bias=bias_sb[:, 3:4], scale=-0.5)
        nc.scalar.dma_start(res, w1(S_res))

    nc.compile()
    return nc


def _host_prep(x, w, b):
    """Build per-core input maps."""
    import ml_dtypes

    x = np.ascontiguousarray(np.asarray(x, np.float32))
    w = np.asarray(w, np.float32)
    b = np.asarray(b, np.float32)

    w2m = np.zeros((C, 24), np.float32)
    for j in range(K):
        for f in range(F):
            w2m[:, j * 8 + f] = -2.0 * w[j, :, f]
    wh = w2m.astype(ml_dtypes.bfloat16)
    wlo = (w2m - wh.astype(np.float32)).astype(ml_dtypes.bfloat16)
    wcat = np.concatenate([wh, wlo], axis=1)              # [C, 48] bf16

    nW = (w ** 2).sum(1)                                  # [K, F]
    biasc = np.zeros((C, 4), np.float32)
    for q in range(NQ):
        for f in range(F):
            for j in range(K):
                biasc[q * 8 + f, j] = nW[j, f]
            biasc[q * 8 + f, 3] = b[f]

    in_maps = []
    for r in range(NCORES):
        x4 = x[r * NB:(r + 1) * NB]                       # [NB,T,C]
        flat = x4.reshape(TL, C)
        xT = np.ascontiguousarray(flat.T)                 # [C, TL] fp32
        xhh = xT.astype(ml_dtypes.bfloat16)
        xll = (xT - xhh.astype(np.float32)).astype(ml_dtypes.bfloat16)
        nS = np.einsum("tc,tc->t", flat, flat).astype(np.float32)
        nsr = np.ones((C, JW), np.float32)
        for q in range(NQ):
            lo = q * FD
            hi = min(TL, lo + FD + 2)
            nsr[q * 8:(q + 1) * 8, 0:hi - lo] = nS[lo:hi][None, :]
        in_maps.append({
            "xh": xhh, "xl": xll, "wcat": wcat, "nsr": nsr, "biasc": biasc,
        })
    return in_maps


def _assemble(results):
    out = np.empty((B, P, F), np.float32)
    for r in range(NCORES):
        resr = results[r]["res"]                          # [128, 512]
        arr = resr.reshape(NQ, 8, FD)                     # [q, f, p_lo]
        for f in range(F):
            series = arr[:, f, :].reshape(TL).reshape(NB, T)
            out[r * NB:(r + 1) * NB, :, f] = series[:, :P]
    return out


def kernel(x, w, b):
    from concourse.bass_utils import run_bass_kernel_spmd

    if "nc" not in _cache:
        _cache["nc"] = _build_program()
    nc = _cache["nc"]
    in_maps = _host_prep(x, w, b)
    out = run_bass_kernel_spmd(nc, in_maps, core_ids=list(range(NCORES)))
    return _assemble(out.results)


if __name__ == "__main__":
    rng = np.random.default_rng(0)
    x = rng.standard_normal((B, T, C), dtype=np.float32)
    w = (rng.standard_normal((K, C, F)) * 0.1).astype(np.float32)
    b = np.zeros((F,), np.float32)
    o = kernel(x, w, b)
    print("kernel ran, out shape", o.shape, float(np.abs(o).sum()))


# revision 18
# speedup vs baseline: 1.2782x; 1.1172x over previous
"""DWA-CNN (DTW-aligned CNN) Trainium2 kernel.

Problem: x[32,2048,128], w[3,128,8], b[8] -> out[32,2046,8]
out[b,p,f] = relu(b[f] + sum of dots along the DTW-optimal path between
window x[b,p:p+3,:] and filter w[:,:,f]).

Strategy (8 cores, pure data parallel over batch, 4 batches/core):
- Host pre-transposes x to [C=128, 8192] per core, split into an exact
  bf16 hi/lo pair (xh + xl == x in fp32), plus -2w as a bf16 hi/lo pair
  and fp32 row norms nsr / weight norms+bias biasc.
- x loaded as 16 [C,1024] pieces alternating sync/gpsimd DMA queues;
  per 512-pos block: 3 bf16 matmuls (wh*xh + wh*xl + wl*xh in fp32
  PSUM == exact fp32 -2*dots).
- PSUM evacuated per block by ScalarE/VectorE (alternating) into a
  [24, 8192] staging tile; 48 small SBUF->SBUF DMAs (rotating over the
  3 DMA-capable engines, issued incrementally as blocks finish)
  scatter rows into the [(q*8+f) partition, j*JW+pos] layout.
- E = Gm + nS via one broadcast 3-window TT on VectorE; D = sqrt(E+nW)
  in-place on ScalarE (activation tables preloaded early).
- DTW DP exploits D>=0: at cell (2,2) the diagonal always wins (ties
  break diag-first), deleting the KA/LA/X1 machinery; c13/c31 are only
  needed for the LB/LC masks.  ~16 fused two-window ops on VectorE.
- Final relu(-0.5*acc + b) on ScalarE; result [128,512] DMAed out; host
  re-assembles [32,2046,8].
"""
import numpy as np

B, T, C, K, F = 32, 2048, 128, 3, 8
P = T - K + 1            # 2046
NCORES = 8
NB = B // NCORES         # batches per core
TL = NB * T              # 8192 positions per core
FD = 512
NQ = TL // FD            # 16 q blocks
JW = FD + 4              # per-j stride in the concatenated arrays

_cache = {}


def _build_program():
    import concourse.tile as tile
    from concourse import bacc, mybir

    f32 = mybir.dt.float32
    bf16 = mybir.dt.bfloat16
    u32 = mybir.dt.uint32
    Alu = mybir.AluOpType
    Act = mybir.ActivationFunctionType

    nc = bacc.Bacc(
        "TRN2",
        target_bir_lowering=False,
        debug=False,
        enable_asserts=False,
        num_devices=NCORES,
    )

    xh = nc.dram_tensor("xh", [C, TL], bf16, kind="ExternalInput").ap()
    xl = nc.dram_tensor("xl", [C, TL], bf16, kind="ExternalInput").ap()
    wcat = nc.dram_tensor("wcat", [C, 48], bf16, kind="ExternalInput").ap()
    nsr = nc.dram_tensor("nsr", [C, JW], f32, kind="ExternalInput").ap()
    biasc = nc.dram_tensor("biasc", [C, 4], f32, kind="ExternalInput").ap()
    res = nc.dram_tensor("res", [C, FD], f32, kind="ExternalOutput").ap()

    from contextlib import ExitStack

    with tile.TileContext(nc) as tc, ExitStack() as ctx:
        const = ctx.enter_context(tc.tile_pool(name="const", bufs=1))
        xin = ctx.enter_context(tc.tile_pool(name="xin", bufs=1))
        psum = ctx.enter_context(tc.tile_pool(name="psum", bufs=6,
                                              space="PSUM"))
        stage = ctx.enter_context(tc.tile_pool(name="stage", bufs=1))
        arrs = ctx.enter_context(tc.tile_pool(name="arrs", bufs=1))
        work = ctx.enter_context(tc.tile_pool(name="work", bufs=1))

        dmaengs = [nc.sync, nc.scalar, nc.gpsimd]
        _dmac = [0]

        def dma3(dst, src):
            e = dmaengs[_dmac[0] % 3]
            _dmac[0] += 1
            e.dma_start(dst, src)

        wcat_sb = const.tile([C, 48], bf16, tag="wcat")
        bias_sb = const.tile([C, 4], f32, tag="bias")
        nsr_sb = const.tile([C, JW], f32, tag="nsr")
        nc.scalar.dma_start(wcat_sb[:], wcat)
        nc.scalar.dma_start(bias_sb[:], biasc)

        # x pieces: [C, 1024] each, 8 per tensor, alternating sync/gpsimd
        NP = 8
        PW = TL // NP        # 1024
        xh_t = []
        xl_t = []
        for i in range(NP):
            th = xin.tile([C, PW], bf16, tag=f"xh{i}", name=f"xh{i}")
            tl_ = xin.tile([C, PW], bf16, tag=f"xl{i}", name=f"xl{i}")
            xh_t.append(th)
            xl_t.append(tl_)
        for i in range(NP):
            sl = slice(i * PW, (i + 1) * PW)
            (nc.sync if i % 2 == 0 else nc.gpsimd).dma_start(
                xh_t[i][:], xh[:, sl])
            (nc.gpsimd if i % 2 == 0 else nc.sync).dma_start(
                xl_t[i][:], xl[:, sl])
        nc.gpsimd.dma_start(nsr_sb[:], nsr)

        # staging for all 16 q blocks (fp32 rows j*8+f)
        stg = stage.tile([24, TL + 4], f32, tag="stg")

        # scattered arrays [partition q*8+f, j*JW + pos]
        Gm_all = arrs.tile([C, K * JW], f32, tag="gm")
        Dj_all = arrs.tile([C, K * JW], f32, tag="dj")

        # halo tails for q=15 (positions beyond TL) default 0; engine APs
        # need 32-aligned partition base, so cover q=12..15 (q<15 halos are
        # later overwritten by their scatter DMAs)
        for j in range(K):
            nc.vector.memset(Gm_all[96:128, j * JW + FD:j * JW + FD + 2], 0.0)

        wh = wcat_sb[:, 0:24]
        wl = wcat_sb[:, 24:48]

        # per-block matmul + evac + incremental scatter
        for b in range(NQ):
            pc = b // 2              # x piece index
            xs = slice((b % 2) * FD, (b % 2 + 1) * FD)
            ps = psum.tile([24, FD], f32, tag="ps", name=f"ps{b}")
            nc.tensor.matmul(ps[:], wh, xh_t[pc][:, xs], start=True, stop=False)
            nc.tensor.matmul(ps[:], wh, xl_t[pc][:, xs], start=False, stop=False)
            nc.tensor.matmul(ps[:], wl, xh_t[pc][:, xs], start=False, stop=True)
            dst = stg[:, b * FD:(b + 1) * FD]
            if b % 2 == 0:
                nc.scalar.copy(dst, ps[:])
            else:
                nc.vector.tensor_scalar_add(dst, ps[:], 0.0)
            # scatter for block b-1 (needs halo = first 2 cols of block b)
            if b > 0:
                q = b - 1
                for j in range(K):
                    dma3(Gm_all[q * 8:(q + 1) * 8, j * JW:j * JW + FD + 2],
                         stg[j * 8:(j + 1) * 8, q * FD:(q + 1) * FD + 2])
        q = NQ - 1
        for j in range(K):
            dma3(Gm_all[q * 8:(q + 1) * 8, j * JW:j * JW + FD],
                 stg[j * 8:(j + 1) * 8, q * FD:(q + 1) * FD])

        V = nc.vector
        TT = V.tensor_tensor
        CP = V.copy_predicated

        # E = Gm + nS (broadcast nsr over the 3 j-blocks), one fused op
        HW2 = FD + 2
        ev = Dj_all[:].rearrange("p (j e) -> p j e", j=K)[:, :, 0:HW2]
        gv = Gm_all[:].rearrange("p (j e) -> p j e", j=K)[:, :, 0:HW2]
        nv = nsr_sb[:, 0:HW2].unsqueeze(1)
        nv.ap[1] = [0, K]
        TT(ev, gv, nv, Alu.add)

        # D = sqrt(E + nW_j), in place
        for j in range(K):
            sl = slice(j * JW, j * JW + HW2)
            nc.scalar.activation(Dj_all[:, sl], Dj_all[:, sl], Act.Sqrt,
                                 bias=bias_sb[:, j:j + 1])

        # work tile slots
        (S_c12, S_c21, S_c13, S_c31, S_c22, S_mn2, S_mn3, S_c23, S_c32,
         S_mbcD, S_s10, S_s01, S_s20, S_s02, S_U, S_X2, S_XV1, S_XV2,
         S_ACCM, S_res) = range(20)
        NW = 20
        W = work.tile([C, NW * FD], f32, tag="W")
        M = work.tile([C, 6 * FD], u32, tag="M")
        M_KB, M_KC, M_LB, M_LC, M_KD, M_LD = range(6)

        def w1(s):
            return W[:, s * FD:(s + 1) * FD]

        def m1(s):
            return M[:, s * FD:(s + 1) * FD]

        def win2(ap2d, off_a, off_b, n=FD):
            v = ap2d[:, off_a:off_a + n].unsqueeze(1)
            v.ap[1] = [off_b - off_a, 2]
            return v

        def wwin(sa, sb):
            return win2(W[:], sa * FD, sb * FD)

        def mwin(sa, sb):
            return win2(M[:], sa * FD, sb * FD)

        def dwin(ia, ja, ib, jb):
            return win2(Dj_all[:], ja * JW + ia, jb * JW + ib)

        def gwin(ia, ja, ib, jb):
            return win2(Gm_all[:], ja * JW + ia, jb * JW + ib)

        def dd(i, j):
            return Dj_all[:, j * JW + i:j * JW + i + FD]

        def gg(i, j):
            return Gm_all[:, j * JW + i:j * JW + i + FD]

        # g-sums first (only need Gm; overlap ACT sqrt)
        TT(wwin(S_s10, S_s01), gwin(1, 0, 0, 1), gwin(0, 0, 0, 0), Alu.add)
        TT(wwin(S_s20, S_s02), gwin(2, 0, 0, 2), wwin(S_s10, S_s01), Alu.add)
        TT(w1(S_U), gg(1, 1), gg(0, 0), Alu.add)
        # X2 default (ACT); X3 lives in the s02 slot in-place
        nc.scalar.copy(w1(S_X2), w1(S_U))

        # cost chain.  c11 = D00; diag always wins at (2,2) since D >= 0.
        TT(wwin(S_c12, S_c21), dwin(0, 1, 1, 0), dwin(0, 0, 0, 0), Alu.add)
        TT(w1(S_c22), dd(1, 1), dd(0, 0), Alu.add)
        TT(wwin(S_c13, S_c31), dwin(0, 2, 2, 0), wwin(S_c12, S_c21), Alu.add)
        TT(wwin(S_mn2, S_mn3), wwin(S_c12, S_c21), wwin(S_c22, S_c22), Alu.min)
        TT(wwin(S_c23, S_c32), dwin(1, 2, 2, 1), wwin(S_mn2, S_mn3), Alu.add)
        TT(mwin(M_KB, M_KC), wwin(S_c12, S_c21), wwin(S_c22, S_c22), Alu.is_le)
        TT(mwin(M_LB, M_LC), wwin(S_c22, S_c31), wwin(S_c13, S_c22), Alu.is_le)
        TT(w1(S_mbcD), w1(S_c32), w1(S_c23), Alu.min)
        TT(mwin(M_KD, M_LD), wwin(S_c22, S_c32), wwin(S_mbcD, S_c23),
           Alu.is_le)

        # X2 = KC ? s10 : (LC ? s20 : U);  X3 = KB ? s01 : (LB ? U : s02)
        CP(wwin(S_X2, S_s02), mwin(M_LC, M_LB), wwin(S_s20, S_U))
        CP(wwin(S_X2, S_s02), mwin(M_KC, M_KB), wwin(S_s10, S_s01))
        TT(wwin(S_XV1, S_XV2), gwin(2, 1, 1, 2), wwin(S_X2, S_s02), Alu.add)
        # X4 = KD ? U : (LD ? XV1 : XV2), built in place in XV2
        CP(w1(S_XV2), m1(M_LD), w1(S_XV1))
        CP(w1(S_XV2), m1(M_KD), w1(S_U))
        TT(w1(S_ACCM), gg(2, 2), w1(S_XV2), Alu.add)

        nc.scalar.activation(
            w1(S_res), w1(S_ACCM), Act.Relu, bias=bias_sb[:, 3:4], scale=-0.5)
        nc.sync.dma_start(res, w1(S_res))

    nc.compile()
    return nc


def _host_prep(x, w, b):
    """Build per-core input maps."""
    import ml_dtypes

    x = np.ascontiguousarray(np.asarray(x, np.float32))
    w = np.asarray(w, np.float32)
    b = np.asarray(b, np.float32)

    w2m = np.zeros((C, 24), np.float32)
    for j in range(K):
        for f in range(F):
            w2m[:, j * 8 + f] = -2.0 * w[j, :, f]
    wh = w2m.astype(ml_dtypes.bfloat16)
    wlo = (w2m - wh.astype(np.float32)).astype(ml_dtypes.bfloat16)
    wcat = np.concatenate([wh, wlo], axis=1)              # [C, 48] bf16

    nW = (w ** 2).sum(1)                                  # [K, F]
    biasc = np.zeros((C, 4), np.float32)
    for q in range(NQ):
        for f in range(F):
            for j in range(K):
                biasc[q * 8 + f, j] = nW[j, f]
            biasc[q * 8 + f, 3] = b[f]

    in_maps = []
    for r in range(NCORES):
        x4 = x[r * NB:(r + 1) * NB]                       # [NB,T,C]
        flat = x4.reshape(TL, C)
        xT = np.ascontiguousarray(flat.T)                 # [C, TL] fp32
        xhh = xT.astype(ml_dtypes.bfloat16)
        xll = (xT - xhh.astype(np.float32)).astype(ml_dtypes.bfloat16)
        nS = np.einsum("tc,tc->t", flat, flat).astype(np.float32)
        nsr = np.ones((C, JW), np.float32)
        for q in range(NQ):
            lo = q * FD
            hi = min(TL, lo + FD + 2)
            nsr[q * 8:(q + 1) * 8, 0:hi - lo] = nS[lo:hi][None, :]
        in_maps.append({
            "xh": xhh, "xl": xll, "wcat": wcat, "nsr": nsr, "biasc": biasc,
        })
    return in_maps


def _assemble(results):
    out = np.empty((B, P, F), np.float32)
    for r in range(NCORES):
        resr = results[r]["res"]                          # [128, 512]
        arr = resr.reshape(NQ, 8, FD)                     # [q, f, p_lo]
        for f in range(F):
            series = arr[:, f, :].reshape(TL).reshape(NB, T)
            out[r * NB:(r + 1) * NB, :, f] = series[:, :P]
    return out


def kernel(x, w, b):
    from concourse.bass_utils import run_bass_kernel_spmd

    if "nc" not in _cache:
        _cache["nc"] = _build_program()
    nc = _cache["nc"]
    in_maps = _host_prep(x, w, b)
    out = run_bass_kernel_spmd(nc, in_maps, core_ids=list(range(NCORES)))
    return _assemble(out.results)


if __name__ == "__main__":
    rng = np.random.default_rng(0)
    x = rng.standard_normal((B, T, C), dtype=np.float32)
    w = (rng.standard_normal((K, C, F)) * 0.1).astype(np.float32)
    b = np.zeros((F,), np.float32)
    o = kernel(x, w, b)
    print("kernel ran, out shape", o.shape, float(np.abs(o).sum()))
